# revision 12
# baseline (speedup 1.0000x reference)
"""Bass/Trainium2 kernel for nn_BBoxDetectionLoss (YOLO-style bbox detection loss).

Strategy (pure data parallel over 8 NeuronCores, 4 images per core):
  The loss decomposes as
    noobj = 0.5 * (sum_all softplus(obj_pred) - sum_resp softplus(obj_pred)) / n_neg
    obj   =        sum_resp softplus(-obj_pred) / n_pos
          =       (sum_resp softplus(obj_pred) - sum_resp obj_pred) / n_pos
    coord = 5 *    sum_resp |bbox_pred - target|^2 / n_pos
  where "resp" is at most 24 cells per image (one per gt box, deduped last-wins).
  So the only dense work is a softplus-sum over the obj channel; the responsible
  cells are fetched with one indirect (gather) DMA per core and everything else
  is tiny per-box arithmetic.

  Each core emits 5 partial sums (dense softplus, resp softplus, resp obj,
  coord sq-err, n_pos) packed into an 8-float vector.  The cross-core sum and
  the final normalization (a dozen scalar flops) happen on the host during the
  gather/unshard step -- a device AllReduce of 32 bytes costs a ~10us collective
  floor plus a wait for the slowest-launched core (~40us observed), so it is
  strictly worse than summing eight 8-float vectors host-side.

  Device schedule (per core):
    - sync ring  (HWDGE): bbox + const loads (tiny, first), then the 9 MB
      prediction stream as NCHUNK chunk DMAs so SDMA saturates HBM immediately.
    - gpsimd (SWDGE): the indirect gather of the 96 responsible cells --
      independent of the big HWDGE stream so it never queues behind it.
    - ACT: per-chunk Exp -> Ln(1+x) with accumulate; the two tiny stage-A
      activations (Ln for tw/th, softplus for gathered obj) are spliced
      between late chunks, where their deps are ready and ACT is otherwise
      idle waiting on DMA.
    - DVE: box-target math in a box-per-partition [96, *] layout (one
      partition per (image, box)): per-op cost is fixed-overhead-bound, the
      gather offsets come out directly as [96, 1] (no transpose DMA), and the
      masked per-box sums reduce across partitions inside the final matmul.
    - PE: broadcast trick for the duplicate-cell kill mask (cross-partition
      compare via transpose + broadcast matmuls), plus the final [128, 8] x
      ones partition-reduction.
"""

import math
import os
import sys

import numpy as np

for _p in ("/opt/trn_rl_repo",):
    if _p not in sys.path:
        sys.path.insert(0, _p)

import concourse.bass as bass
import concourse.tile as tile
from concourse import bacc, mybir
from concourse.bass_utils import run_bass_kernel_spmd

F32 = mybir.dt.float32
I32 = mybir.dt.int32

N_CORES = 8
B, H, W, A, C = 32, 112, 112, 9, 5
NBOX = 24
BL = B // N_CORES                     # images per core = 4
NB = BL * NBOX                        # boxes per core = 96
CELLS_L = BL * H * W * A              # 451584 cells per core
ELEMS_L = CELLS_L * C                 # 2257920 f32 per core
P = 128
FPL = ELEMS_L // P                    # 17640 elements per partition
CELLS_PP = CELLS_L // P               # 3528 cells per partition
# uneven chunk sizes (cells per partition): big chunks while the stream is
# the bottleneck, tiny final chunk so the post-stream softplus tail is short
CHUNK_CELLS = [
    int(x) for x in os.environ.get(
        "K_CHUNKS", "546,546,546,546,546,441,252,105"
    ).split(",")
]
assert sum(CHUNK_CELLS) == CELLS_PP
NCHUNK = len(CHUNK_CELLS)
# which HWDGE ring issues each chunk: 0 = sync (SP), 1 = scalar (ACT)
RING_SPLIT = os.environ.get("K_RING", "0")  # "0"=all sync, "alt"=alternate
TOT_CELLS = B * H * W * A             # 3612672 (for n_neg)
NCOL = NCHUNK + 4                     # out tile: NCHUNK accum cols + 4 partial cols

LAMBDA_COORD = 5.0
LAMBDA_NOOBJ = 0.5

USE_DIV = os.environ.get("K_DIV", "1") == "1"
SPL_TWTH = int(os.environ.get("K_SPL_TWTH", "4"))   # chunk idx to splice tw/th Ln after
SPL_SPP = int(os.environ.get("K_SPL_SPP", "5"))     # chunk idx to splice spp softplus after

# ---- host-side constants ---------------------------------------------------


def _anchors():
    a = []
    for s in (32, 64, 128):
        for r in (0.5, 1.0, 2.0):
            a.append(
                (
                    np.float32(s * math.sqrt(r) / 224.0),
                    np.float32(s / math.sqrt(r) / 224.0),
                )
            )
    return np.array(a, np.float32)  # [9, 2]


# const tensor layout, [96, KC] f32 (one row per (image i, box j), i-major):
#   [0:9)     AW9     anchor widths
#   [9:18)    AH9     anchor heights
#   [18:27)   AWAH9   aw*ah (f32 product, bit-identical to reference's)
#   [27:36)   IOTA9   float(a)
#   [36:45)   RAW9    1/aw (f32)
#   [45:54)   RAH9    1/ah (f32)
#   [54:150)  JGT96   [q, q'] mask: 1.0 iff same image and j' > j
#   [150:246) I96     identity row (one-hot(q))
#   [246:247) BASE96  per-box flat-offset base = i * H*W*A
#   [247:248) LARGEQ  1e6 * (q+1), unique tag for invalid boxes in dedup
KC = 248


def _build_const():
    anc = _anchors()
    aw, ah = anc[:, 0], anc[:, 1]
    awah = (aw * ah).astype(np.float32)
    raw = (np.float32(1.0) / aw).astype(np.float32)
    rah = (np.float32(1.0) / ah).astype(np.float32)
    cst = np.zeros((NB, KC), np.float32)
    cst[:, 0:9] = aw
    cst[:, 9:18] = ah
    cst[:, 18:27] = awah
    cst[:, 27:36] = np.arange(9, dtype=np.float32)
    cst[:, 36:45] = raw
    cst[:, 45:54] = rah
    q = np.arange(NB)
    same_img = (q[:, None] // NBOX) == (q[None, :] // NBOX)
    j_gt = (q[None, :] % NBOX) > (q[:, None] % NBOX)
    cst[:, 54:150] = (same_img & j_gt).astype(np.float32)
    cst[:, 150:246] = np.eye(NB, dtype=np.float32)
    cst[:, 246] = (q // NBOX).astype(np.float32) * (H * W * A)
    cst[:, 247] = (q + 1).astype(np.float32) * 1e6
    return cst


# ---- bass program ----------------------------------------------------------

MAGIC = 8388608.0  # 2^23: (x + 2^23) - 2^23 rounds x to nearest integer
SPLIT = 4097.0     # 2^12 + 1: Dekker split constant for f32

_DIV_UID = [0]


def _dtile(sm, shape):
    _DIV_UID[0] += 1
    return sm.tile(shape, F32, name=f"dv{_DIV_UID[0]}", tag=f"dv{_DIV_UID[0]}")


def _two_prod_err(nc, sm, q, qh, ql, bh, bl, b_ap, shape):
    """err = q*b - fl(q*b) exactly (Dekker); returns (p, err) tiles."""
    p = _dtile(sm, shape)
    nc.vector.tensor_tensor(out=p[:], in0=q[:], in1=b_ap, op=mybir.AluOpType.mult)
    e = _dtile(sm, shape)
    t = _dtile(sm, shape)
    nc.vector.tensor_mul(e[:], qh[:], bh[:])
    nc.vector.tensor_sub(e[:], e[:], p[:])
    nc.vector.tensor_mul(t[:], qh[:], bl[:])
    nc.vector.tensor_add(e[:], e[:], t[:])
    nc.vector.tensor_mul(t[:], ql[:], bh[:])
    nc.vector.tensor_add(e[:], e[:], t[:])
    nc.vector.tensor_mul(t[:], ql[:], bl[:])
    nc.vector.tensor_add(e[:], e[:], t[:])
    return p, e


def _dekker_split(nc, sm, x_ap, shape):
    """x = xh + xl with xh having <=12 mantissa bits; exact products follow."""
    c = _dtile(sm, shape)
    nc.vector.tensor_scalar_mul(c[:], x_ap, SPLIT)
    u = _dtile(sm, shape)
    nc.vector.tensor_tensor(out=u[:], in0=c[:], in1=x_ap, op=mybir.AluOpType.subtract)
    xh = _dtile(sm, shape)
    nc.vector.tensor_sub(xh[:], c[:], u[:])
    xl = _dtile(sm, shape)
    nc.vector.tensor_tensor(out=xl[:], in0=x_ap, in1=xh[:], op=mybir.AluOpType.subtract)
    return xh, xl


def _exact_div(nc, sm, a_ap, b_ap, shape):
    """q = RN(a/b) bit-exact (positive a, normal b), matching IEEE f32 divide.

    DVE reciprocal is correctly rounded (verified on HW), so q0 = fl(a*RN(1/b))
    is within ~1 ulp of a/b.  The residual r = a - q0*b is computed exactly via
    Dekker TwoProd (no FMA needed); the Newton correction c = r*rec then has
    ~1e-7-ulp error, and the final f32 add q = fl(q0 + c) performs the correct
    rounding itself.  Verified bit-exact vs numpy f32 divide on 10M samples.
    """
    rec = _dtile(sm, shape)
    nc.vector.reciprocal(rec[:], b_ap)
    q0 = _dtile(sm, shape)
    nc.vector.tensor_tensor(out=q0[:], in0=a_ap, in1=rec[:], op=mybir.AluOpType.mult)

    bh, bl = _dekker_split(nc, sm, b_ap, shape)
    qh, ql = _dekker_split(nc, sm, q0[:], shape)
    p, e = _two_prod_err(nc, sm, q0, qh, ql, bh, bl, b_ap, shape)
    r = _dtile(sm, shape)
    nc.vector.tensor_tensor(out=r[:], in0=a_ap, in1=p[:], op=mybir.AluOpType.subtract)
    nc.vector.tensor_sub(r[:], r[:], e[:])
    nc.vector.tensor_mul(r[:], r[:], rec[:])
    q = _dtile(sm, shape)
    nc.vector.tensor_add(q[:], q0[:], r[:])
    return q


# Force exp and ln onto the single combined ACT table set: strip them from
# every other set (indices preserved; act_func_set_id is positional) so
# Bacc's table-load pass emits one ACT_TABLE_LOAD instead of ping-ponging
# between exp_and_others and natural_log on every chunk (~1.3us per load).
def _patch_act_tables():
    import functools

    import concourse.bacc as _bacc
    import concourse.hw_specs as _hs

    orig = _hs.get_activation_tables

    @functools.cache
    def patched(arch):
        t = {k: set(v) for k, v in orig(arch).items()}
        keep = "natural_log_exp_and_others"
        strip = {mybir.ActivationFunctionType.Exp, mybir.ActivationFunctionType.Ln}
        if keep in t and strip <= t[keep]:
            for k in t:
                if k != keep:
                    t[k] = t[k] - strip
        return t

    _bacc.get_activation_tables = patched


_patch_act_tables()


def _build_nc():
    nc = bacc.Bacc(
        "TRN2", target_bir_lowering=False, debug=False, num_devices=N_CORES
    )

    pred = nc.dram_tensor("pred", [ELEMS_L], F32, kind="ExternalInput")
    bbt = nc.dram_tensor("bb", [NB, 4], F32, kind="ExternalInput")
    cstt = nc.dram_tensor("cst", [NB, KC], F32, kind="ExternalInput")
    partsd = nc.dram_tensor("parts", [P, NCOL], F32, kind="ExternalOutput")

    predv = pred[:].rearrange("(p f) -> p f", p=P)          # [128, 17640]
    gatherv = pred[:].rearrange("(n c) -> n c", c=C)        # [451584, 5]

    with tile.TileContext(nc) as tc:
        with (
            tc.tile_pool(name="big", bufs=NCHUNK) as big,
            tc.tile_pool(name="spp_pool", bufs=2) as spool,
            tc.tile_pool(name="small", bufs=1) as sm,
            tc.tile_pool(name="psum", bufs=1, space="PSUM") as pp,
        ):
            # ---- sync ring: first dense chunk, tiny inputs, rest of stream
            chunks = []
            off = 0
            for i, cells in enumerate(CHUNK_CELLS):
                ch = big.tile([P, cells * C], F32, tag="chunk")
                chunks.append((ch, cells))
                eng = nc.sync
                if RING_SPLIT == "alt" and i % 2 == 1:
                    eng = nc.scalar
                eng.dma_start(
                    out=ch[:], in_=predv[:, off : off + cells * C]
                )
                off += cells * C
                if i == 0:
                    bb = sm.tile([NB, 4], F32)
                    nc.sync.dma_start(out=bb[:], in_=bbt[:])
                    cst = sm.tile([NB, KC], F32)
                    nc.sync.dma_start(out=cst[:], in_=cstt[:])

            # ---- gpsimd: constants for the final pack (no deps, run early)
            rhs = sm.tile([P, NCOL], F32)
            nc.gpsimd.memset(rhs[:], 0.0)
            onesr = sm.tile([1, NB], F32)
            nc.gpsimd.memset(onesr[:], 1.0)

            AW9 = cst[:, 0:9]
            AH9 = cst[:, 9:18]
            AWAH9 = cst[:, 18:27]
            IOTA9 = cst[:, 27:36]
            RAW9 = cst[:, 36:45]
            RAH9 = cst[:, 45:54]
            JGT96 = cst[:, 54:150]
            I96 = cst[:, 150:246]
            BASE96 = cst[:, 246:247]
            LARGEQ = cst[:, 247:248]

            a3 = lambda ap: ap.rearrange("p (i a) -> p i a", a=9)
            wv = bb[:, 2:3]
            hv = bb[:, 3:4]

            # ---------------- stage A, critical path to the gather ----------
            # sx = cx*W, sy = cy*H in one op (W == H == 112)
            sxy = sm.tile([NB, 2], F32)
            nc.vector.tensor_scalar_mul(sxy[:], bb[:, 0:2], float(W))
            # floor via 2^23 round-trip (RN) + correction, then clip to [0, 111]
            gxy = sm.tile([NB, 2], F32)
            corr = sm.tile([NB, 2], F32)
            nc.vector.tensor_scalar(
                gxy[:], sxy[:], MAGIC, -MAGIC,
                op0=mybir.AluOpType.add, op1=mybir.AluOpType.add,
            )
            nc.vector.tensor_tensor(
                out=corr[:], in0=gxy[:], in1=sxy[:], op=mybir.AluOpType.is_gt
            )
            nc.vector.tensor_sub(gxy[:], gxy[:], corr[:])
            nc.vector.tensor_scalar(
                gxy[:], gxy[:], float(W - 1), 0.0,
                op0=mybir.AluOpType.min, op1=mybir.AluOpType.max,
            )

            # IoU against 9 anchors -> best (first max wins).  The quotient must
            # be bit-exact IEEE f32 division: exact ties between anchors decide
            # argmax, and the reference breaks them by first-index.
            t9a = sm.tile([NB, 9], F32)
            t9b = sm.tile([NB, 9], F32)
            w9 = wv.to_broadcast([NB, 1, 9])
            h9 = hv.to_broadcast([NB, 1, 9])
            nc.vector.tensor_tensor(
                out=a3(t9a[:]), in0=w9, in1=a3(AW9), op=mybir.AluOpType.min
            )
            nc.vector.tensor_tensor(
                out=a3(t9b[:]), in0=h9, in1=a3(AH9), op=mybir.AluOpType.min
            )
            nc.vector.tensor_mul(t9a[:], t9a[:], t9b[:])  # inter
            wh = sm.tile([NB, 1], F32)
            nc.vector.tensor_mul(wh[:], wv, hv)
            nc.vector.tensor_tensor(
                out=a3(t9b[:]), in0=wh[:].to_broadcast([NB, 1, 9]),
                in1=a3(AWAH9), op=mybir.AluOpType.add,
            )
            nc.vector.tensor_sub(t9b[:], t9b[:], t9a[:])  # union
            nc.vector.tensor_scalar_add(t9b[:], t9b[:], 1e-16)
            if USE_DIV:
                iou = _exact_div(nc, sm, t9a[:], t9b[:], [NB, 9])
            else:
                iou = sm.tile([NB, 9], F32, name="iou_t", tag="iou_t")
                nc.vector.reciprocal(iou[:], t9b[:])
                nc.vector.tensor_mul(iou[:], iou[:], t9a[:])

            ioumax = sm.tile([NB, 1], F32)
            nc.vector.tensor_reduce(
                ioumax[:], a3(iou[:]), axis=mybir.AxisListType.X,
                op=mybir.AluOpType.max,
            )
            # val = eq ? a : 9  ->  val = eq * (a - 9) + 9 ; best = min(val)
            nc.vector.tensor_tensor(
                out=a3(t9a[:]), in0=a3(iou[:]),
                in1=ioumax[:].to_broadcast([NB, 1, 9]),
                op=mybir.AluOpType.is_equal,
            )
            nc.vector.tensor_scalar_add(t9b[:], IOTA9, -9.0)
            nc.vector.tensor_mul(t9b[:], t9a[:], t9b[:])
            nc.vector.tensor_scalar_add(t9b[:], t9b[:], 9.0)
            best = sm.tile([NB, 1], F32)
            nc.vector.tensor_reduce(
                best[:], a3(t9b[:]), axis=mybir.AxisListType.X,
                op=mybir.AluOpType.min,
            )

            # cell id (within image) and flat gather offsets
            cellf = sm.tile([NB, 1], F32)
            nc.vector.tensor_scalar_mul(cellf[:], gxy[:, 1:2], float(W))
            nc.vector.tensor_add(cellf[:], cellf[:], gxy[:, 0:1])
            nc.vector.tensor_scalar_mul(cellf[:], cellf[:], float(A))
            nc.vector.tensor_add(cellf[:], cellf[:], best[:])
            offf = sm.tile([NB, 1], F32)
            nc.vector.tensor_add(offf[:], cellf[:], BASE96)
            offi = sm.tile([NB, 1], I32)
            nc.vector.tensor_copy(offi[:], offf[:])

            # ---- indirect gather (SWDGE): one offset per partition row -----
            g96 = sm.tile([NB, C], F32)
            nc.gpsimd.indirect_dma_start(
                out=g96[:],
                out_offset=None,
                in_=gatherv,
                in_offset=bass.IndirectOffsetOnAxis(ap=offi[:], axis=0),
            )

            # ---------------- stage A, off the gather path ------------------
            # targets: tx = sx-gx, ty = sy-gy, tw = ln(w/aw+eps), th = ln(h/ah+eps)
            T96 = sm.tile([NB, 4], F32)
            nc.vector.tensor_sub(T96[:, 0:2], sxy[:], gxy[:])

            # validity: any coord nonzero
            vmax = sm.tile([NB, 1], F32)
            nc.vector.tensor_reduce(
                vmax[:], bb[:], axis=mybir.AxisListType.X,
                op=mybir.AluOpType.max, apply_absolute_value=True,
            )
            valid = sm.tile([NB, 1], F32)
            nc.vector.tensor_scalar(
                valid[:], vmax[:], 0.0, None, op0=mybir.AluOpType.is_gt
            )

            # one-hot select of 1/aw, 1/ah by best anchor
            nc.vector.tensor_tensor(
                out=a3(t9a[:]), in0=a3(IOTA9),
                in1=best[:].to_broadcast([NB, 1, 9]),
                op=mybir.AluOpType.is_equal,
            )
            rawsel = sm.tile([NB, 1], F32)
            rahsel = sm.tile([NB, 1], F32)
            nc.vector.tensor_mul(t9b[:], t9a[:], RAW9)
            nc.vector.tensor_reduce(
                rawsel[:], a3(t9b[:]), axis=mybir.AxisListType.X,
                op=mybir.AluOpType.add,
            )
            nc.vector.tensor_mul(t9b[:], t9a[:], RAH9)
            nc.vector.tensor_reduce(
                rahsel[:], a3(t9b[:]), axis=mybir.AxisListType.X,
                op=mybir.AluOpType.add,
            )
            nc.vector.tensor_mul(T96[:, 2:3], wv, rawsel[:])
            nc.vector.tensor_mul(T96[:, 3:4], hv, rahsel[:])
            nc.vector.tensor_scalar_add(T96[:, 2:4], T96[:, 2:4], 1e-16)
            # (the Ln on T96[:, 2:4] runs on ACT, spliced into the chunk loop)

            # ---- dedup via PE broadcast: box q dies if a later valid box of
            # the same image hits the same cell.  Invalid boxes get a unique
            # huge cell tag so they never match anything.
            iv = sm.tile([NB, 1], F32)
            nc.vector.tensor_scalar(
                iv[:], valid[:], -1.0, 1.0,
                op0=mybir.AluOpType.mult, op1=mybir.AluOpType.add,
            )
            cellv = sm.tile([NB, 1], F32)
            nc.vector.tensor_mul(iv[:], iv[:], LARGEQ)
            nc.vector.tensor_add(cellv[:], cellf[:], iv[:])
            psA = pp.tile([1, NB], F32, name="psA", tag="psA")
            nc.tensor.matmul(psA[:], lhsT=cellv[:], rhs=I96, start=True, stop=True)
            cvT = sm.tile([1, NB], F32)
            nc.vector.tensor_copy(cvT[:], psA[:])
            psB = pp.tile([NB, NB], F32, name="psB", tag="psB")
            nc.tensor.matmul(psB[:], lhsT=onesr[:], rhs=cvT[:], start=True, stop=True)
            E = sm.tile([NB, NB], F32)
            e3 = lambda ap: ap.rearrange("p (i j) -> p i j", i=1)
            nc.vector.tensor_tensor(
                out=e3(E[:]), in0=e3(psB[:]),
                in1=cellv[:].to_broadcast([NB, 1, NB]),
                op=mybir.AluOpType.is_equal,
            )
            nc.vector.tensor_mul(E[:], E[:], JGT96)
            dead = sm.tile([NB, 1], F32)
            nc.vector.tensor_reduce(
                dead[:], E[:], axis=mybir.AxisListType.X,
                op=mybir.AluOpType.max,
            )
            live = sm.tile([NB, 1], F32)
            nc.vector.tensor_mul(live[:], valid[:], dead[:])
            nc.vector.tensor_sub(live[:], valid[:], live[:])
            nc.vector.tensor_copy(rhs[0:NB, NCHUNK + 3 : NCHUNK + 4], live[:])

            # ---------------- stage B: dense softplus over obj channel -----
            # softplus(x) = ln(exp(x) + 1); exp and ln share one ACT table set.
            # Per-chunk accumulators land directly in columns of the out tile
            # (the Tile scheduler hoists these by readiness, so splice order
            # in the ACT queue is not load-bearing).
            spp = sm.tile([NB, 1], F32)
            for i, (ch, cells) in enumerate(chunks):
                sp = spool.tile([P, cells], F32, tag="sp")
                ch4 = ch[:, 4::5]
                nc.scalar.activation(
                    sp[:], ch4, mybir.ActivationFunctionType.Exp
                )
                nc.scalar.activation(
                    sp[:], sp[:], mybir.ActivationFunctionType.Ln, bias=1.0,
                    accum_out=rhs[:, i : i + 1],
                )
                if i == SPL_TWTH:
                    nc.scalar.activation(
                        T96[:, 2:4], T96[:, 2:4], mybir.ActivationFunctionType.Ln
                    )
                if i == SPL_SPP:
                    nc.scalar.activation(
                        spp[:], g96[:, 4:5], mybir.ActivationFunctionType.Exp
                    )
                    nc.scalar.activation(
                        spp[:], spp[:], mybir.ActivationFunctionType.Ln, bias=1.0
                    )

            # ---- masked per-box values straight into the out tile ----------
            # cols [0:NCHUNK) = dense accs; then live*softplus(obj), live*obj,
            # live*coordsq, live
            nc.vector.tensor_tensor(
                out=rhs[0:NB, NCHUNK : NCHUNK + 1], in0=spp[:], in1=live[:],
                op=mybir.AluOpType.mult,
            )
            nc.vector.tensor_tensor(
                out=rhs[0:NB, NCHUNK + 1 : NCHUNK + 2], in0=g96[:, 4:5], in1=live[:],
                op=mybir.AluOpType.mult,
            )
            d = sm.tile([NB, 4], F32)
            nc.vector.tensor_tensor(
                out=d[:], in0=g96[:, 0:4], in1=T96[:],
                op=mybir.AluOpType.subtract,
            )
            nc.vector.tensor_mul(d[:], d[:], d[:])
            cb = sm.tile([NB, 1], F32)
            nc.vector.tensor_reduce(
                cb[:], d[:], axis=mybir.AxisListType.X, op=mybir.AluOpType.add
            )
            nc.vector.tensor_tensor(
                out=rhs[0:NB, NCHUNK + 2 : NCHUNK + 3], in0=cb[:], in1=live[:],
                op=mybir.AluOpType.mult,
            )

            # single store of the [128, NCOL] partials; the host sums the
            # partition axis during gather/unshard
            nc.sync.dma_start(out=partsd[:], in_=rhs[:])

    nc.compile()
    return nc


_NC_CACHE = None


def _get_nc():
    global _NC_CACHE
    if _NC_CACHE is None:
        _NC_CACHE = _build_nc()
    return _NC_CACHE


def kernel_with_results(predictions, bboxes, **run_kwargs):
    predictions = np.ascontiguousarray(predictions, dtype=np.float32)
    bboxes = np.ascontiguousarray(bboxes, dtype=np.float32)
    assert predictions.shape == (B, H, W, A, C)
    assert bboxes.shape == (B, NBOX, 4)

    cst = _build_const()
    in_maps = []
    for c in range(N_CORES):
        shard_p = predictions[c * BL : (c + 1) * BL].reshape(-1)
        shard_b = bboxes[c * BL : (c + 1) * BL].reshape(NB, 4)
        in_maps.append({"pred": shard_p, "bb": shard_b, "cst": cst})

    nc = _get_nc()
    res = run_bass_kernel_spmd(nc, in_maps, core_ids=list(range(N_CORES)), **run_kwargs)

    # gather/unshard: the per-core, per-partition partial sums add across the
    # data-parallel shards; normalization is a handful of scalar flops.
    parts = np.zeros(NCOL, np.float64)
    for c in range(N_CORES):
        arr = np.asarray(res.results[c]["parts"], dtype=np.float64).reshape(P, NCOL)
        parts += arr.sum(axis=0)
    dense_s = parts[0:NCHUNK].sum()
    sub_s, o_s, coord_s, npos = parts[NCHUNK : NCHUNK + 4]
    n_pos = max(npos, 1.0)
    n_neg = max(float(TOT_CELLS) - npos, 1.0)
    coord = LAMBDA_COORD * coord_s / n_pos
    obj = (sub_s - o_s) / n_pos
    noobj = LAMBDA_NOOBJ * (dense_s - sub_s) / n_neg
    total = coord + obj + noobj
    out = np.array([total, coord, obj, noobj, 0.0], np.float32)
    return out, res


def kernel(predictions, bboxes):
    out, _ = kernel_with_results(predictions, bboxes)
    return out
